# revision 1
# baseline (speedup 1.0000x reference)
"""Trainium2 Bass kernel for a 2-layer GNN message-passing encoder.

Math (per layer):  out = segment_mean(x[src] * w, dst) + x @ Wr.T
with w = typew(src,dst) * edge_weight, run twice (Wr1 then Wr2).

Device strategy (8 NeuronCores, SPMD single program):
  - Nodes padded to 50176 = 8 cores * 49 ranks * 128; core c owns the
    contiguous 6272-node range [c*6272, (c+1)*6272), i.e. 49 windows of
    128 nodes. Edges are assigned to the core owning their dst.
  - Per 128-node window, the weighted segment-mean is computed as a sum of
    one-hot matmuls accumulated in PSUM: for each 128-edge tile,
      S[e, n] = (dst_rel[e] == n) * w'[e]
    where w' = typew * edge_weight * 1/max(count(dst),1) is folded on host
    so PSUM directly accumulates the mean. S is built host-side as a dense
    fp8e4m3 tensor and STREAMED from DRAM over HWDGE (an on-device DVE
    build stalls SWDGE: DVE perf-mode ops hold the shared SBUF port pair
    that GPSIMD needs for gather-descriptor generation). The root linear
    x @ Wr.T is one more (fp16) matmul accumulated into the same PSUM bank.
  - Layer-1 x[src] rows are gathered ON HOST into slot order and streamed
    contiguously at byte rate. Layer-2 h[src] rows are fetched with the
    SWDGE dma_gather custom instruction (fp16, 256B rows, descriptor-rate
    bound) from the AllGathered h. int16 gather indices can't span 50176
    rows, so each window's edges are split into lo (row < 24576) and hi
    classes; the hi gather uses a base-shifted view of the source. Pad
    slots use idx=0 with an all-zero one-hot column.
  - Between layers, per-core h slices (fp16) are AllGathered to rebuild the
    full gather source for layer 2. The AllGather is CHUNKED by rank range
    (gather sources use a chunk-major node permutation so each chunk's
    output is contiguous, a BIR requirement) and emitted as soon as the
    producing groups have stored h, overlapping the tail of layer-1
    compute. The first l2_prefetch groups' lo-class layer-2 gathers are
    emitted between chunks 1 and 2 so the later collectives' waits don't
    head-of-line block them on the Pool FIFO. hT (layer-2 root lhsT, fp16)
    is rebuilt per chunk with an HWDGE DMA-transpose from the h slice.
  - Queue discipline: loads issue from nc.sync (SP), stores and transposes
    from nc.scalar (ACT), gathers and collectives from Pool — each queue's
    program order then matches its dependency-readiness order.

Host does index/structure work plus two data-movement precomputes (the
dense S image and the layer-1 gather); all O(E*D) and O(N*D*D) float math
runs on device.
"""

import sys
from contextlib import ExitStack
from dataclasses import dataclass, field

import numpy as np

sys.path.insert(0, "/opt/trn_rl_repo")

import concourse.bacc as bacc  # noqa: E402
import concourse.mybir as mybir  # noqa: E402
import concourse.tile as tile  # noqa: E402
from concourse.bass_utils import run_bass_kernel_spmd  # noqa: E402

D = 128
SAME_W = 0.3
CROSS_W = 1.0


@dataclass
class Cfg:
    n_nodes: int = 50000
    n_cores: int = 8
    ranks_per_core: int = 49
    group: int = 4            # windows per gather batch
    # lo/hi src split at permuted row 192*128 = 24576 — aligned with the end
    # of AllGather chunk 1 so layer-2 lo-class gathers can start as soon as
    # chunks 0+1 have landed (overlapping the tail of layer-1 compute).
    # split_rank=160 with bounds (2,5,9,11,13) benched WORSE (675 vs 645 us):
    # the bigger hi class (59% of descs, gated on the full collective)
    # outweighs the earlier lo unblock.
    split_rank: int = 192
    # SWDGE ring: carveout/64B = descs per engine ring; a gather of T tiles
    # needs T*8+1 descs per engine and must fit well under the ring size.
    # NOTE: dma_scratch is charged per SBUF partition; 65536 + 4-deep pools
    # exceeds the 208KB/partition budget at full size. gather_tiles_max=64
    # also needs the bigger ring. Validated configuration: 32768/32/3/3.
    dma_scratch: int = 32768
    gather_tiles_max: int = 32
    n_queues: int = 4
    single_packet: bool = False
    gbufs: int = 3            # gather-tile pool depth (group lookahead)
    sbufs: int = 3            # S-tile pool depth
    # layer-2 groups whose lo-class gathers are emitted between AllGather
    # chunks 1 and 2 on the Pool FIFO: they only need h rows < split (chunks
    # 0+1), so this unblocks them from the later collectives' head-of-line
    # wait and pulls their descriptor work under the layer-1 tail. Bounded
    # by gbufs-1 useful depth (the WAR on the rotating gather buffers).
    l2_prefetch: int = 2
    # one-hot S source: "stream" = host-built dense S streamed from DRAM
    # (DMA-only; avoids the DVE-perf-mode port lock that starves SWDGE
    # descriptor generation), "dve" = fused on-device DVE build
    s_mode: str = "stream"
    # dtype of the streamed dense S ("float8e4" halves its 60MB/iter byte
    # traffic; w' in [0,1] quantizes to ~3% which stays well under the 2e-2
    # correctness gate)
    s_dtype: str = "float8e4"
    # layer-1 x[src] rows host-gathered into slot order and streamed as one
    # contiguous tensor (byte-rate) instead of per-edge SWDGE gathers
    # (descriptor-rate, ~60ns/desc/engine); layer 2 still gathers h on device
    l1_host_gather: bool = True
    # AllGather chunk boundaries, in units of "after group index" (exclusive
    # prefix of groups whose ranks the chunk covers). None = single
    # collective after layer 1 completes.
    coll_bounds: tuple = (3, 6, 9, 11, 13)

    @property
    def npc(self) -> int:           # nodes per core (padded)
        return self.ranks_per_core * 128

    @property
    def npad(self) -> int:
        return self.n_cores * self.npc

    @property
    def split(self) -> int:
        return self.split_rank * 128

    def chunk_ranges(self, n_groups: int, groups: list) -> list:
        """AllGather chunks as (after_group_idx, r0, r1) rank ranges."""
        if self.coll_bounds is None:
            bounds = [n_groups]
        else:
            bounds = sorted({min(b, n_groups) for b in self.coll_bounds} | {n_groups})
        out = []
        prev = 0
        for b in bounds:
            if b > prev:
                out.append((b - 1, groups[prev][0], groups[b - 1][-1] + 1))
                prev = b
        return out

    def perm_rows(self, groups: list) -> np.ndarray:
        """Gather-source layout: permrow[node] — chunk-major ordering so each
        chunked AllGather output [all cores' rows of chunk k] is contiguous."""
        chunks = self.chunk_ranges(len(groups), groups)
        npc, nc_ = self.npc, self.n_cores
        i = np.arange(self.npad, dtype=np.int64)
        c = i // npc
        r = (i % npc) >> 7
        p = i & 127
        perm = np.empty(self.npad, np.int64)
        off = 0
        for (_g, r0, r1) in chunks:
            sz = (r1 - r0) * 128
            m = (r >= r0) & (r < r1)
            perm[m] = off + c[m] * sz + (r[m] - r0) * 128 + p[m]
            off += nc_ * sz
        return perm


@dataclass
class Plan:
    cfg: Cfg
    TL: np.ndarray        # [ranks_per_core] lo-tile capacity per local window
    TH: np.ndarray        # [ranks_per_core] hi-tile capacity per local window
    base_lo: np.ndarray   # [ranks_per_core] tile index of window's lo run
    base_hi: np.ndarray
    groups: list = field(default_factory=list)  # list of lists of local window ids
    # gather instructions: (idx_col_start, slot_tile_start, n_tiles, is_hi, gi)
    ginstrs: list = field(default_factory=list)
    idx_cols: int = 0     # total int16 columns in the gather-index buffer

    @property
    def n_tiles(self) -> int:
        return int(self.TL.sum() + self.TH.sum())


def _make_plan(cfg: Cfg, cnt_lo: np.ndarray, cnt_hi: np.ndarray) -> Plan:
    """cnt_lo/cnt_hi: [n_cores, ranks_per_core] per-window edge counts."""
    RPC = cfg.ranks_per_core
    TL = np.ceil(cnt_lo.max(axis=0) / 128).astype(np.int64)
    TH = np.ceil(cnt_hi.max(axis=0) / 128).astype(np.int64)
    groups = [list(range(q, min(q + cfg.group, RPC))) for q in range(0, RPC, cfg.group)]
    base_lo = np.zeros(RPC, np.int64)
    base_hi = np.zeros(RPC, np.int64)
    t = 0
    runs = []  # (tile_start, n_tiles, is_hi, group_idx) per (group, class) run
    for gi, grp in enumerate(groups):
        lo0 = t
        for wl in grp:
            base_lo[wl] = t
            t += TL[wl]
        runs.append((lo0, t - lo0, False, gi))
        hi0 = t
        for wl in grp:
            base_hi[wl] = t
            t += TH[wl]
        runs.append((hi0, t - hi0, True, gi))
    # chunk runs into gather instructions; each instruction's idx block is
    # 128B-aligned (64 int16 columns) in the index buffer (HW requirement).
    ginstrs = []
    col = 0
    for (t0, n_run, is_hi, gi) in runs:
        done = 0
        while done < n_run:
            n = min(cfg.gather_tiles_max, n_run - done)
            ginstrs.append((col, t0 + done, n, is_hi, gi))
            col += ((n * 8 + 63) // 64) * 64
            done += n
    return Plan(cfg=cfg, TL=TL, TH=TH, base_lo=base_lo, base_hi=base_hi,
                groups=groups, ginstrs=ginstrs, idx_cols=max(col, 64))


def preprocess(x, edge_index, edge_weight, Wr1, Wr2, cell_len, cfg: Cfg):
    """Host-side index/structure prep. Returns (plan, in_maps)."""
    RPC = cfg.ranks_per_core
    src = np.asarray(edge_index[0], dtype=np.int64)
    dst = np.asarray(edge_index[1], dtype=np.int64)
    ew = np.asarray(edge_weight, dtype=np.float32)
    cl = int(np.asarray(cell_len))
    x = np.asarray(x, dtype=np.float32)

    tw = np.where((src > cl) == (dst > cl), SAME_W, CROSS_W).astype(np.float32)
    cnt = np.bincount(dst, minlength=cfg.n_nodes).astype(np.float32)
    inv = (1.0 / np.maximum(cnt, 1.0)).astype(np.float32)
    wfin = tw * ew * inv[dst]

    # gather-source row layout (chunk-major so chunked AllGather outputs are
    # contiguous); src indices below are in permuted rows
    groups0 = [list(range(q, min(q + cfg.group, RPC))) for q in range(0, RPC, cfg.group)]
    perm = cfg.perm_rows(groups0)
    psrc = perm[src]

    g = dst >> 7                      # global window id
    klass = (psrc >= cfg.split).astype(np.int64)   # 0 = lo, 1 = hi
    n_wg = cfg.n_cores * RPC
    gid = g * 2 + klass
    counts = np.bincount(gid, minlength=n_wg * 2)
    cnt_lo = counts[0::2].reshape(cfg.n_cores, RPC)
    cnt_hi = counts[1::2].reshape(cfg.n_cores, RPC)
    plan = _make_plan(cfg, cnt_lo, cnt_hi)

    # slot position of each edge: sorted by (window, class, src) — the src
    # minor key makes each run's gather addresses ascending (HBM-friendly)
    order = np.lexsort((psrc, klass, g))
    gid_s = gid[order]
    gid_starts = np.zeros(n_wg * 2 + 1, np.int64)
    np.cumsum(counts, out=gid_starts[1:])
    pos = np.arange(len(src), dtype=np.int64) - gid_starts[gid_s]

    gs = g[order]
    core_e = gs // RPC
    wl_e = gs - core_e * RPC
    kl_e = klass[order]
    tile_base = np.where(kl_e == 0, plan.base_lo[wl_e], plan.base_hi[wl_e])
    n_slots = plan.n_tiles * 128
    slot = core_e * n_slots + tile_base * 128 + pos

    src_s = psrc[order]
    idx_val = np.where(kl_e == 0, src_s, src_s - cfg.split).astype(np.int16)
    rel_val = (dst[order] - (gs << 7)).astype(np.int64)
    w_val = wfin[order]

    total = cfg.n_cores * n_slots
    idx_slot = np.zeros(total, np.int16)
    idx_slot[slot] = idx_val
    # per-slot one-hot metadata (dst_rel, w'), fp32 for the DVE scalar ports;
    # pad slots get dst_rel=-1 (never matches iota 0..127) and w'=0.
    rel_slot = np.full(total, -1.0, np.float32)
    rel_slot[slot] = rel_val.astype(np.float32)
    w_slot = np.zeros(total, np.float32)
    w_slot[slot] = w_val

    nt = plan.n_tiles
    if cfg.l1_host_gather:
        # layer-1 gathered rows, in slot layout [128, nt, 128] matching the
        # device gather output (slot s -> partition s%128, tile s//128)
        psrc_slot = np.zeros(cfg.n_cores * n_slots, np.int64)
        psrc_slot[slot] = src_s
        xg1 = None  # built per-core below to bound memory

    if cfg.s_mode == "stream":
        # dense one-hot S, built host-side: S[core][e, tile, dst_rel] = w'
        np_sdt = mybir.dt.np(getattr(mybir.dt, cfg.s_dtype))
        s_dense = np.zeros((cfg.n_cores, 128, nt, 128), np_sdt)
        e_sl = slot % 128
        t_sl = (slot // 128) % nt
        c_sl = slot // (nt * 128)
        s_dense[c_sl, e_sl, t_sl, rel_val] = w_val.astype(np_sdt)
    else:
        # smeta[core]: [128, 2*nt] f32 — cols [0:nt] = dst_rel, [nt:2nt] = w'
        rel_pt = rel_slot.reshape(cfg.n_cores, nt, 128).transpose(0, 2, 1)
        w_pt = w_slot.reshape(cfg.n_cores, nt, 128).transpose(0, 2, 1)
        smeta = np.concatenate([rel_pt, w_pt], axis=2)  # [cores, 128, 2*nt]

    # device-layout constants; x16 is stored in the permuted gather layout
    xnat = np.zeros((cfg.npad, D), np.float16)
    xnat[: cfg.n_nodes] = x.astype(np.float16)
    xpad16 = np.empty_like(xnat)
    xpad16[perm] = xnat
    w1t = np.ascontiguousarray(np.asarray(Wr1, np.float16).T)
    w2t = np.ascontiguousarray(np.asarray(Wr2, np.float16).T)
    iota16 = np.tile(np.arange(128, dtype=np.float16), (128, 1))

    in_maps = []
    for c in range(cfg.n_cores):
        idx_c = idx_slot[c * n_slots : (c + 1) * n_slots]
        g16 = np.zeros((16, plan.idx_cols), np.int16)
        for (c0, t0, n_t, _hi, _gi) in plan.ginstrs:
            g16[:, c0 : c0 + n_t * 8] = idx_c[t0 * 128 : (t0 + n_t) * 128].reshape(
                -1, 16
            ).T
        gidx = np.ascontiguousarray(np.tile(g16, (8, 1)))  # [128, idx_cols]
        xT = np.ascontiguousarray(
            xnat[c * cfg.npc : (c + 1) * cfg.npc].T
        )  # [D, npc] f16
        m = {
            "xT16": xT,
            "w1t": w1t,
            "w2t": w2t,
            "gidx": gidx,
        }
        if not cfg.l1_host_gather:
            m["x16"] = xpad16
        if cfg.s_mode == "stream":
            m["sden"] = s_dense[c].reshape(128, nt * 128)
        else:
            m["smeta"] = np.ascontiguousarray(smeta[c])
            m["iota16"] = iota16
        if cfg.l1_host_gather:
            rows = xpad16[psrc_slot[c * n_slots : (c + 1) * n_slots]]  # [ns,128]
            m["xg1"] = np.ascontiguousarray(
                rows.reshape(nt, 128, D).transpose(1, 0, 2).reshape(128, nt * D)
            )
        in_maps.append(m)
    return plan, in_maps


def build_program(plan: Plan, dbg_layers=(0, 1), dbg_gather=True,
                  dbg_coll=True, dbg_compute=True, dbg_sbuild=True, repeat=1):
    cfg = plan.cfg
    RPC = cfg.ranks_per_core
    dt = mybir.dt
    f32, f16, i16 = dt.float32, dt.float16, dt.int16
    n_tiles = plan.n_tiles

    nc = bacc.Bacc(
        "TRN2",
        target_bir_lowering=False,
        debug=False,
        num_devices=cfg.n_cores,
        dynamic_dma_scratch_size=cfg.dma_scratch,
        num_swdge_queues=cfg.n_queues,
    )
    if not cfg.l1_host_gather:
        x16_d = nc.dram_tensor("x16", [cfg.npad, D], f16, kind="ExternalInput")
    xT16_d = nc.dram_tensor("xT16", [D, cfg.npc], f16, kind="ExternalInput")
    w1t_d = nc.dram_tensor("w1t", [D, D], f16, kind="ExternalInput")
    w2t_d = nc.dram_tensor("w2t", [D, D], f16, kind="ExternalInput")
    gidx_d = nc.dram_tensor("gidx", [128, plan.idx_cols], i16, kind="ExternalInput")
    if cfg.l1_host_gather:
        xg1_d = nc.dram_tensor("xg1", [128, n_tiles * D], f16, kind="ExternalInput")
    stream_s = cfg.s_mode == "stream"
    sdt = getattr(dt, cfg.s_dtype)
    if stream_s:
        sden_d = nc.dram_tensor("sden", [128, n_tiles * 128], sdt,
                                kind="ExternalInput")
    else:
        smeta_d = nc.dram_tensor("smeta", [128, 2 * n_tiles], f32,
                                 kind="ExternalInput")
        iota_d = nc.dram_tensor("iota16", [128, 128], f16, kind="ExternalInput")
    out_d = nc.dram_tensor("out", [cfg.npc, D], f32, kind="ExternalOutput")
    h_slice_d = nc.dram_tensor("h_slice", [cfg.npc, D], f16)
    h_full_d = nc.dram_tensor("h_full", [cfg.npad, D], f16, addr_space="Shared")

    Copy = mybir.ActivationFunctionType.Copy
    is_eq, mult = mybir.AluOpType.is_equal, mybir.AluOpType.mult

    two_layers = len(dbg_layers) > 1

    # collective chunk boundaries: after group gi, AllGather ranks [r0, r1)
    # into the contiguous h_full block at chunk_off (chunk-major layout,
    # matching Cfg.perm_rows)
    chunks = cfg.chunk_ranges(len(plan.groups), plan.groups)
    chunk_of_group = {}  # group idx -> (r0, r1, h_full_row_offset)
    off = 0
    for (gend, r0, r1) in chunks:
        chunk_of_group[gend] = (r0, r1, off)
        off += cfg.n_cores * (r1 - r0) * 128

    with tile.TileContext(nc) as tc, ExitStack() as ctx:
        const = ctx.enter_context(tc.tile_pool(name="const", bufs=1))
        gpool = ctx.enter_context(tc.tile_pool(name="g", bufs=cfg.gbufs))
        spool = ctx.enter_context(tc.tile_pool(name="s", bufs=cfg.sbufs))
        hpool = ctx.enter_context(tc.tile_pool(name="hw", bufs=4))
        psum_w = ctx.enter_context(tc.tile_pool(name="pw", bufs=6, space="PSUM"))

        xT_s = const.tile([D, cfg.npc], f16)
        nc.sync.dma_start(xT_s[:], xT16_d[:, :])
        w1t_s = const.tile([D, D], f16)
        nc.sync.dma_start(w1t_s[:], w1t_d[:, :])
        w2t_s = const.tile([D, D], f16)
        nc.sync.dma_start(w2t_s[:], w2t_d[:, :])
        gidx_s = const.tile([128, plan.idx_cols], i16)
        nc.sync.dma_start(gidx_s[:], gidx_d[:, :])
        if not stream_s:
            iota_s = const.tile([128, 128], f16)
            nc.sync.dma_start(iota_s[:], iota_d[:, :])
            smeta_s = const.tile([128, 2 * n_tiles], f32)
            nc.sync.dma_start(smeta_s[:], smeta_d[:, :])

        if two_layers:
            hT_s = const.tile([D, cfg.npc], f16)
        else:
            hT_s = None

        max_grp_tiles = max(
            int(sum(plan.TL[wl] + plan.TH[wl] for wl in grp)) for grp in plan.groups
        )
        grp_instrs = [[] for _ in plan.groups]
        for inst in plan.ginstrs:
            grp_instrs[inst[4]].append(inst)
        qn = [0]

        def _emit_gathers(gi, gt, grp_t0, src_lo, src_hi, classes):
            # narrow source views: the lo view only overlaps the AllGather
            # chunks covering rows < split, so layer-2 lo gathers wait only
            # on those chunks
            for (c0, t0, n_t, is_hi, _gi) in grp_instrs[gi]:
                if is_hi not in classes:
                    continue
                off = t0 - grp_t0
                nc.gpsimd.dma_gather(
                    gt[:, off : off + n_t, :],
                    src_hi if is_hi else src_lo,
                    gidx_s[:, c0 : c0 + n_t * 8],
                    n_t * 128,
                    n_t * 128,
                    D,
                    queue_num=qn[0],
                    single_packet=cfg.single_packet,
                )
                qn[0] = (qn[0] + 1) % cfg.n_queues

        prefetched = {}  # L2 group idx -> (gt, sg) with lo gathers emitted

        for layer in [l for _ in range(repeat) for l in dbg_layers]:
            lhsT_root = xT_s if layer == 0 else hT_s
            wt_s = w1t_s if layer == 0 else w2t_s

            for gi, grp in enumerate(plan.groups):
                grp_t0 = int(plan.base_lo[grp[0]])
                n_gt = int(sum(plan.TL[wl] + plan.TH[wl] for wl in grp))
                gw = len(grp)
                if n_gt == 0:
                    gt = None
                    sg = None
                elif layer != 0 and gi in prefetched:
                    # lo gathers already emitted during the layer-1 collective
                    # window; finish with the hi class only
                    gt, sg = prefetched.pop(gi)
                    _emit_gathers(gi, gt, grp_t0,
                                  h_full_d[0 : cfg.split, :],
                                  h_full_d[cfg.split :, :], classes=(True,))
                else:
                    gt = gpool.tile([128, max_grp_tiles, D], f16, tag="g")
                    sg = spool.tile([128, max_grp_tiles, D],
                                    sdt if stream_s else f16, tag="s")
                    if not dbg_gather:
                        nc.vector.memset(gt[:], 0.5)
                    elif layer == 0 and cfg.l1_host_gather:
                        # layer-1 rows were gathered on host: one contiguous
                        # byte-rate stream instead of per-edge descriptors
                        nc.sync.dma_start(
                            gt[:, 0:n_gt, :],
                            xg1_d[:, grp_t0 * D : (grp_t0 + n_gt) * D],
                        )
                    else:
                        if layer == 0:
                            src_lo = x16_d[0 : cfg.split, :]
                            src_hi = x16_d[cfg.split :, :]
                        else:
                            src_lo = h_full_d[0 : cfg.split, :]
                            src_hi = h_full_d[cfg.split :, :]
                        _emit_gathers(gi, gt, grp_t0, src_lo, src_hi,
                                      classes=(False, True))
                if sg is not None and stream_s:
                    if dbg_sbuild:
                        nc.sync.dma_start(
                            sg[:, 0:n_gt, :],
                            sden_d[:, grp_t0 * 128 : (grp_t0 + n_gt) * 128],
                        )
                elif sg is not None and dbg_sbuild:
                    # on-device one-hot build: one fused DVE op per tile
                    for o in range(n_gt):
                        t_abs = grp_t0 + o
                        nc.vector.tensor_scalar(
                            sg[:, o, :],
                            iota_s[:],
                            smeta_s[:, t_abs : t_abs + 1],
                            smeta_s[:, n_tiles + t_abs : n_tiles + t_abs + 1],
                            is_eq,
                            mult,
                        )

                is_last_layer = layer == dbg_layers[-1]
                if is_last_layer:
                    stage = hpool.tile([128, cfg.group, D], f32, tag="ostage")
                else:
                    stage = hpool.tile([128, cfg.group, D], f16, tag="hstage")
                if not dbg_compute:
                    if gt is not None:
                        nc.vector.tensor_copy(stage[:, 0, :], gt[:, 0, :])
                    else:
                        nc.vector.memset(stage[:], 0.0)
                for wi, wl in enumerate(grp):
                    if not dbg_compute:
                        continue
                    tiles = [int(plan.base_lo[wl]) + i for i in range(int(plan.TL[wl]))]
                    tiles += [int(plan.base_hi[wl]) + i for i in range(int(plan.TH[wl]))]
                    pw = psum_w.tile([128, D], f32, tag="pw")
                    nc.tensor.matmul(
                        pw[:],
                        lhsT_root[:, wl * 128 : (wl + 1) * 128],
                        wt_s[:],
                        start=True,
                        stop=(len(tiles) == 0),
                    )
                    for j, tg in enumerate(tiles):
                        o = tg - grp_t0
                        nc.tensor.matmul(
                            pw[:],
                            sg[:, o, :],
                            gt[:, o, :],
                            start=False,
                            stop=(j == len(tiles) - 1),
                        )
                    nc.scalar.activation(stage[:, wi, :], pw[:], Copy)
                # flush this group's staging to DRAM. Stores issue from the
                # ACT (scalar) HWDGE queue: their deps are the stage copies
                # just ahead of them there, so the SP queue stays free for
                # loads (sden streams / hT transposes) whose deps resolve
                # much earlier — avoids FIFO head-of-line blocking.
                r0, r1 = grp[0] * 128, (grp[-1] + 1) * 128
                if is_last_layer:
                    nc.scalar.dma_start(
                        out_d[r0:r1, :].rearrange("(w p) d -> p w d", p=128),
                        stage[:, 0:gw, :],
                    )
                else:
                    nc.scalar.dma_start(
                        h_slice_d[r0:r1, :].rearrange("(w p) d -> p w d", p=128),
                        stage[:, 0:gw, :],
                    )

            # chunked AllGather + hT rebuild, emitted AFTER all of this
            # layer's gathers so the collectives (whose deps are whole
            # compute chains) never head-of-line block gather issue on the
            # Pool FIFO; each chunk still fires as soon as its producing
            # groups have stored h.
            if layer == 0 and two_layers:
                for gi in sorted(chunk_of_group):
                    cr0, cr1, coff = chunk_of_group[gi]
                    a, b = cr0 * 128, cr1 * 128
                    csz = cfg.n_cores * (b - a)
                    if dbg_coll:
                        nc.gpsimd.collective_compute(
                            "AllGather",
                            mybir.AluOpType.bypass,
                            replica_groups=[list(range(cfg.n_cores))],
                            ins=[h_slice_d[a:b, :]],
                            outs=[h_full_d[coff : coff + csz, :]],
                        )
                    else:
                        nc.sync.dma_start(
                            h_full_d[coff : coff + (b - a), :], h_slice_d[a:b, :]
                        )
                    nc.scalar.dma_start_transpose(hT_s[:, a:b], h_slice_d[a:b, :])
                    # once the chunks covering h rows < split are out, slot
                    # the first layer-2 groups' lo gathers into the Pool FIFO
                    # ahead of the remaining collectives (which wait on the
                    # whole layer-1 tail). Tile allocation order here matches
                    # what the layer-2 loop would do, so pool rotation is
                    # unchanged.
                    if (dbg_gather and cfg.l2_prefetch > 0
                            and coff + csz == cfg.split):
                        for pgi in range(min(cfg.l2_prefetch, len(plan.groups))):
                            pgrp = plan.groups[pgi]
                            pn_gt = int(sum(plan.TL[w] + plan.TH[w] for w in pgrp))
                            if pn_gt == 0:
                                continue
                            pgt = gpool.tile([128, max_grp_tiles, D], f16, tag="g")
                            psg = spool.tile([128, max_grp_tiles, D],
                                             sdt if stream_s else f16, tag="s")
                            _emit_gathers(pgi, pgt, int(plan.base_lo[pgrp[0]]),
                                          h_full_d[0 : cfg.split, :],
                                          h_full_d[cfg.split :, :],
                                          classes=(False,))
                            prefetched[pgi] = (pgt, psg)

    nc.compile()
    return nc


_CACHE: dict = {}


def _get_program(plan: Plan):
    key = (
        plan.cfg.n_nodes,
        plan.cfg.n_cores,
        plan.cfg.ranks_per_core,
        plan.cfg.group,
        tuple(plan.TL.tolist()),
        tuple(plan.TH.tolist()),
    )
    if key not in _CACHE:
        _CACHE[key] = build_program(plan)
    return _CACHE[key]


def kernel(x, edge_index, edge_weight, Wr1, Wr2, cell_len):
    cfg = Cfg()
    assert x.shape == (cfg.n_nodes, D)
    plan, in_maps = preprocess(x, edge_index, edge_weight, Wr1, Wr2, cell_len, cfg)
    nc = _get_program(plan)
    res = run_bass_kernel_spmd(nc, in_maps, list(range(cfg.n_cores)))
    out = np.concatenate([res.results[c]["out"] for c in range(cfg.n_cores)], axis=0)
    return np.ascontiguousarray(out[: cfg.n_nodes]).astype(np.float32)



# revision 32
# speedup vs baseline: 13.1933x; 13.1933x over previous
"""Trainium2 Bass kernel for a 2-layer GNN message-passing encoder.

Math (per layer):  out = segment_mean(x[src] * w, dst) + x @ Wr.T
with w = typew(src,dst) * edge_weight, run twice (Wr1 then Wr2).

Device strategy (8 NeuronCores, SPMD single program), v3 "push-local L2":

  Layer 1 (pull, dst-partitioned edges): core c owns the contiguous
  6272-node dst range. Per 128-node dst window, the weighted segment-mean
  is a chain of one-hot matmuls accumulated in PSUM:
      S[e, n] = (dst_rel[e] == n) * w'[e],   w' = typew * ew / max(cnt,1)
  where S tiles are built ON DEVICE by one fused DVE tensor_scalar per
  tile from a tiny per-slot metadata stream (rel, w'), and the x[src]
  rows are gathered ON HOST into slot order and streamed as fp8 at byte
  rate (no per-edge descriptors). The root linear x @ W1.T is one more
  matmul into the same PSUM chain (lhsT = resident xT).  h goes to DRAM
  (h_slice, fp16) and hT is rebuilt once with a DMA transpose.

  Layer 2 (push-local, src-partitioned edges): each core computes
  partial aggregates for ALL 392 dst windows from the edges whose SRC it
  owns — h[src] rows are gathered from the core's OWN h_slice with the
  SWDGE dma_gather (local rows < 6272, so int16 indices need no class
  split), messages scatter into per-window PSUM via the same masked
  one-hot matmuls, and partials are stored to DRAM in a chunk-major
  window order.  A chunked ReduceScatter (the only collective) then sums
  partials across cores, delivering each core exactly its own 6272 rows.
  The root linear h @ W2.T is precomputed per own-window into a resident
  r2 buffer (PE, from hT) and added to each ReduceScatter chunk on DVE
  before the final store.  Edge tiles are aligned per OCT (8 windows) so
  per-window padding never hits the gather descriptor stream; windows'
  matmul pieces cover the union of tile spans across cores, with
  per-core masking folded into the per-piece S metadata (zero columns
  contribute nothing).

  Queue discipline: SP carries loads + the ReduceScatters + rs reloads,
  ACT carries stage copies / stores / the hT transpose, Pool carries the
  SWDGE gathers — each queue's program order matches its readiness order
  so nothing head-of-line blocks.

Host does index/structure work plus one data-movement precompute (the
layer-1 fp8 gather image); all O(E*D) and O(N*D*D) float math runs on
device.
"""

import sys
from contextlib import ExitStack
from dataclasses import dataclass, field

import numpy as np

sys.path.insert(0, "/opt/trn_rl_repo")

import concourse.bacc as bacc  # noqa: E402
import concourse.mybir as mybir  # noqa: E402
import concourse.tile as tile  # noqa: E402
from concourse.bass_utils import run_bass_kernel_spmd  # noqa: E402

D = 128
SAME_W = 0.3
CROSS_W = 1.0


@dataclass
class Cfg:
    n_nodes: int = 50000
    n_cores: int = 8
    ranks_per_core: int = 49
    group: int = 2            # L1 windows per stream batch
    oct: int = 8              # L2 windows per tile-aligned batch
    # L2 ReduceScatter chunk bounds, exclusive prefix of own-rank index
    rs_bounds: tuple = (16, 32, 44)
    # dtype of the exchanged partial aggregates (fp8 halves store+RS bytes;
    # quantization error averages out across the 8-way reduction)
    partial_dtype: str = "float16"
    # fraction of S one-hot builds issued on GPSIMD (Pool) instead of DVE —
    # Pool is idle in L1 and ~60% idle in L2; its per-op cost is ~2.9x DVE's,
    # so a balanced split shortens the build critical path
    pool_build_l1: int = 4   # 1 of every pool_build_l1 builds goes to Pool
    pool_build_l2: int = 8
    # dtype of the host-gathered layer-1 x[src] image
    xg_dtype: str = "float8e4"
    dma_scratch: int = 32768
    gather_tiles_max: int = 32
    n_queues: int = 4
    single_packet: bool = False
    gbufs: int = 4            # L1 xg / L2 gather tile pool depth
    sbufs: int = 3            # S-tile pool depth

    @property
    def npc(self) -> int:
        return self.ranks_per_core * 128

    @property
    def npad(self) -> int:
        return self.n_cores * self.npc

    def rs_chunks(self) -> list:
        """[(r0, r1)] own-rank ranges per ReduceScatter chunk."""
        bounds = sorted({min(b, self.ranks_per_core) for b in self.rs_bounds}
                        | {self.ranks_per_core})
        out = []
        prev = 0
        for b in bounds:
            if b > prev:
                out.append((prev, b))
                prev = b
        return out

    def worder(self) -> np.ndarray:
        """worder[global_window] = processing position (chunk-major:
        chunk k, then owner core, then rank)."""
        RPC = self.ranks_per_core
        chunks = self.rs_chunks()
        pos = np.empty(self.n_cores * RPC, np.int64)
        p = 0
        for (r0, r1) in chunks:
            for c in range(self.n_cores):
                for r in range(r0, r1):
                    pos[c * RPC + r] = p
                    p += 1
        return pos

    def oct_groups(self):
        """L2 batches: consecutive worder positions, never straddling a
        (chunk, core) run. Returns (octs, meta) with meta[i] =
        (chunk_idx, owner_core, rank_start, n_windows)."""
        octs, meta = [], []
        p = 0
        for k, (r0, r1) in enumerate(self.rs_chunks()):
            for c in range(self.n_cores):
                r = r0
                while r < r1:
                    n = min(self.oct, r1 - r)
                    octs.append(list(range(p, p + n)))
                    meta.append((k, c, r, n))
                    p += n
                    r += n
        return octs, meta


@dataclass
class Plan:
    cfg: Cfg
    # layer 1
    T1: np.ndarray        # [RPC] tiles per local dst window
    base1: np.ndarray     # [RPC] first tile of window
    nt1: int = 0
    # layer 2
    T2: np.ndarray = None        # [n_octs] tiles per oct
    base2: np.ndarray = None     # [n_octs] first tile of oct
    nt2: int = 0
    octs: list = field(default_factory=list)     # list of [worder positions]
    pieces: list = field(default_factory=list)   # (wpos, tile_abs, oct_idx)
    ginstrs: list = field(default_factory=list)  # (col, tile_abs, n_tiles, oct)
    idx_cols: int = 0
    npieces: int = 0


def _make_plan(cfg: Cfg, cnt1: np.ndarray, cnt2oct: np.ndarray,
               spans: np.ndarray, octs: list) -> Plan:
    """cnt1: [cores, RPC] L1 per-window counts.
    cnt2oct: [cores, n_octs] L2 per-oct counts.
    spans: [n_windows(worder-ordered), 2] union tile span per window
    (absolute tile indices, computed by caller)."""
    T1 = np.ceil(cnt1.max(axis=0) / 128).astype(np.int64)
    base1 = np.concatenate([[0], np.cumsum(T1)[:-1]])
    nt1 = int(T1.sum())

    n_octs = cnt2oct.shape[1]
    T2 = np.ceil(cnt2oct.max(axis=0) / 128).astype(np.int64)
    base2 = np.concatenate([[0], np.cumsum(T2)[:-1]])
    nt2 = int(T2.sum())

    pieces = []
    for oi, opos in enumerate(octs):
        for wpos in opos:
            lo, hi = spans[wpos]
            for t in range(lo, hi):
                pieces.append((wpos, int(t), oi))

    ginstrs = []
    col = 0
    for oi in range(n_octs):
        t0, n_run = int(base2[oi]), int(T2[oi])
        done = 0
        while done < n_run:
            n = min(cfg.gather_tiles_max, n_run - done)
            ginstrs.append((col, t0 + done, n, oi))
            col += ((n * 8 + 63) // 64) * 64
            done += n
    return Plan(cfg=cfg, T1=T1, base1=base1, nt1=nt1, T2=T2, base2=base2,
                nt2=nt2, octs=octs, pieces=pieces, ginstrs=ginstrs,
                idx_cols=max(col, 64), npieces=len(pieces))


def preprocess(x, edge_index, edge_weight, Wr1, Wr2, cell_len, cfg: Cfg):
    """Host-side index/structure prep. Returns (plan, in_maps)."""
    RPC = cfg.ranks_per_core
    NC = cfg.n_cores
    src = np.asarray(edge_index[0], dtype=np.int64)
    dst = np.asarray(edge_index[1], dtype=np.int64)
    ew = np.asarray(edge_weight, dtype=np.float32)
    cl = int(np.asarray(cell_len))
    x = np.asarray(x, dtype=np.float32)

    tw = np.where((src > cl) == (dst > cl), SAME_W, CROSS_W).astype(np.float32)
    cnt = np.bincount(dst, minlength=cfg.n_nodes).astype(np.float32)
    inv = (1.0 / np.maximum(cnt, 1.0)).astype(np.float32)
    wfin = tw * ew * inv[dst]

    g_dst = dst >> 7                    # global dst window
    core1 = g_dst // RPC                # L1 owner (by dst)
    wl1 = g_dst - core1 * RPC
    cnt1 = np.bincount(core1 * RPC + wl1, minlength=NC * RPC).reshape(NC, RPC)

    # ---- layer 2 structure (by src core, chunk-major window order) ----
    worder = cfg.worder()               # global window -> position
    wpos_e = worder[g_dst]
    core2 = src // cfg.npc              # L2 owner (by src)
    src_local = src - core2 * cfg.npc

    octs, _oct_meta = cfg.oct_groups()
    n_octs = len(octs)
    nw = NC * RPC
    oct_id_of = np.empty(nw, np.int64)       # worder position -> oct index
    oct_first = np.empty(n_octs, np.int64)   # oct -> first worder position
    for oi, opos in enumerate(octs):
        oct_id_of[opos] = oi
        oct_first[oi] = opos[0]
    oct_e = oct_id_of[wpos_e]
    cnt2oct = np.bincount(core2 * n_octs + oct_e,
                          minlength=NC * n_octs).reshape(NC, n_octs)

    # per-core slot position within oct: sort edges by (core2, wpos, src_local)
    order2 = np.lexsort((src_local, wpos_e, core2))
    key2 = (core2 * n_octs + oct_e)[order2]
    starts2 = np.zeros(NC * n_octs + 1, np.int64)
    np.cumsum(np.bincount(key2, minlength=NC * n_octs), out=starts2[1:])
    pos2 = np.arange(len(src)) - starts2[key2]

    # union tile span per window position across cores
    # per (core, wpos): start/end offsets within oct
    wcnt = np.bincount(core2 * nw + wpos_e, minlength=NC * nw).reshape(NC, nw)
    wend = np.cumsum(wcnt, axis=1)      # per core: cumulative end over wpos
    # reset cumsum at oct boundaries: offset within oct
    oct_start_w = oct_first[oct_id_of[np.arange(nw)]]
    base_at_oct = np.where(oct_start_w > 0, wend[:, oct_start_w - 1], 0)
    w_off_end = wend - base_at_oct      # end offset within oct per (core, wpos)
    w_off_start = w_off_end - wcnt

    plan0_T2 = np.ceil(cnt2oct.max(axis=0) / 128).astype(np.int64)
    plan0_base2 = np.concatenate([[0], np.cumsum(plan0_T2)[:-1]])
    has = wcnt > 0
    start_t = np.where(has, w_off_start // 128, np.iinfo(np.int64).max)
    end_t = np.where(has, (w_off_end + 127) // 128, 0)
    lo_w = start_t.min(axis=0)
    hi_w = end_t.max(axis=0)
    spans = np.zeros((nw, 2), np.int64)
    touched = has.any(axis=0)
    oi_w = oct_id_of
    spans[touched, 0] = plan0_base2[oi_w[touched]] + lo_w[touched]
    spans[touched, 1] = plan0_base2[oi_w[touched]] + hi_w[touched]

    plan = _make_plan(cfg, cnt1, cnt2oct, spans, octs)
    nt1, nt2 = plan.nt1, plan.nt2

    # ---- layer 1 slots (per dst core) ----
    order1 = np.lexsort((src, g_dst))
    gid1_s = g_dst[order1]
    starts1 = np.zeros(NC * RPC + 1, np.int64)
    np.cumsum(np.bincount(gid1_s, minlength=NC * RPC), out=starts1[1:])
    pos1 = np.arange(len(src)) - starts1[gid1_s]
    core1_s = core1[order1]
    wl1_s = wl1[order1]
    slot1 = core1_s * (nt1 * 128) + plan.base1[wl1_s] * 128 + pos1

    rel1 = (dst[order1] & 127).astype(np.float32)
    w1v = wfin[order1]
    total1 = NC * nt1 * 128
    rel1_slot = np.full(total1, -1.0, np.float32)
    rel1_slot[slot1] = rel1
    w1_slot = np.zeros(total1, np.float32)
    w1_slot[slot1] = w1v
    src1_slot = np.zeros(total1, np.int64)
    src1_slot[slot1] = src[order1]

    # smeta1: [cores][128, 2*nt1] f32 (col t = rel, col nt1+t = w')
    rel1_pt = rel1_slot.reshape(NC, nt1, 128).transpose(0, 2, 1)
    w1_pt = w1_slot.reshape(NC, nt1, 128).transpose(0, 2, 1)
    smeta1 = np.concatenate([rel1_pt, w1_pt], axis=2)

    # ---- layer 2 slots (per src core) ----
    oct_s = oct_e[order2]
    slot2 = plan.base2[oct_s] * 128 + pos2     # within-core slot
    core2_s = core2[order2]
    total2_core = nt2 * 128
    rel2_slot = np.zeros((NC, total2_core), np.float32)
    w2_slot = np.zeros((NC, total2_core), np.float32)
    win2_slot = np.full((NC, total2_core), -1, np.int64)
    idx2_slot = np.zeros((NC, total2_core), np.int16)
    rel2_slot[core2_s, slot2] = (dst[order2] & 127).astype(np.float32)
    w2_slot[core2_s, slot2] = wfin[order2]
    win2_slot[core2_s, slot2] = wpos_e[order2]
    idx2_slot[core2_s, slot2] = src_local[order2].astype(np.int16)

    # smeta2: per piece p (wpos, t): rel col masked to this window
    piece_w = np.array([p[0] for p in plan.pieces], np.int64)
    piece_t = np.array([p[1] for p in plan.pieces], np.int64)
    npieces = plan.npieces
    rel2_tiles = rel2_slot.reshape(NC, nt2, 128)
    w2_tiles = w2_slot.reshape(NC, nt2, 128)
    win2_tiles = win2_slot.reshape(NC, nt2, 128)
    # [cores, npieces, 128]
    m = win2_tiles[:, piece_t, :] == piece_w[None, :, None]
    rel_p = np.where(m, rel2_tiles[:, piece_t, :], -1.0)
    w_p = np.where(m, w2_tiles[:, piece_t, :], 0.0)
    smeta2 = np.concatenate(
        [rel_p.transpose(0, 2, 1), w_p.transpose(0, 2, 1)], axis=2
    )  # [cores, 128, 2*npieces]

    # device-layout constants
    np_xdt = mybir.dt.np(getattr(mybir.dt, cfg.xg_dtype))
    xnat = np.zeros((cfg.npad, D), np.float32)
    xnat[: cfg.n_nodes] = x
    x8 = xnat.astype(np_xdt)
    iota16 = np.tile(np.arange(128, dtype=np.float16), (128, 1))
    w1t = np.ascontiguousarray(np.asarray(Wr1, np.float16).T)
    w2t = np.ascontiguousarray(np.asarray(Wr2, np.float16).T)

    in_maps = []
    for c in range(NC):
        # layer-1 gathered rows in slot layout [128, nt1*D] (fp8)
        rows = x8[src1_slot[c * nt1 * 128 : (c + 1) * nt1 * 128]]
        xg1 = np.ascontiguousarray(
            rows.reshape(nt1, 128, D).transpose(1, 0, 2).reshape(128, nt1 * D)
        )
        # gather indices for layer 2
        idx_c = idx2_slot[c]
        g16 = np.zeros((16, plan.idx_cols), np.int16)
        for (c0, t0, n_t, _oi) in plan.ginstrs:
            g16[:, c0 : c0 + n_t * 8] = idx_c[t0 * 128 : (t0 + n_t) * 128].reshape(
                -1, 16
            ).T
        gidx = np.ascontiguousarray(np.tile(g16, (8, 1)))
        xT = np.ascontiguousarray(
            xnat[c * cfg.npc : (c + 1) * cfg.npc].astype(np.float16).T
        )
        in_maps.append({
            "xT16": xT,
            "w1t": w1t,
            "w2t": w2t,
            "iota16": iota16,
            "xg1": xg1,
            "gidx": gidx,
            "smeta1": np.ascontiguousarray(smeta1[c]),
            "smeta2": np.ascontiguousarray(smeta2[c]),
        })
    return plan, in_maps


def build_program(plan: Plan, repeat=1):
    cfg = plan.cfg
    RPC = cfg.ranks_per_core
    NC = cfg.n_cores
    dt = mybir.dt
    f32, f16, i16 = dt.float32, dt.float16, dt.int16
    xdt = getattr(dt, cfg.xg_dtype)
    nt1, nt2, npieces = plan.nt1, plan.nt2, plan.npieces

    nc = bacc.Bacc(
        "TRN2",
        target_bir_lowering=False,
        debug=False,
        num_devices=NC,
        dynamic_dma_scratch_size=cfg.dma_scratch,
        num_swdge_queues=cfg.n_queues,
    )
    xT16_d = nc.dram_tensor("xT16", [D, cfg.npc], f16, kind="ExternalInput")
    w1t_d = nc.dram_tensor("w1t", [D, D], f16, kind="ExternalInput")
    w2t_d = nc.dram_tensor("w2t", [D, D], f16, kind="ExternalInput")
    iota_d = nc.dram_tensor("iota16", [128, 128], f16, kind="ExternalInput")
    xg1_d = nc.dram_tensor("xg1", [128, nt1 * D], xdt, kind="ExternalInput")
    gidx_d = nc.dram_tensor("gidx", [128, plan.idx_cols], i16, kind="ExternalInput")
    smeta1_d = nc.dram_tensor("smeta1", [128, 2 * nt1], f32, kind="ExternalInput")
    smeta2_d = nc.dram_tensor("smeta2", [128, 2 * npieces], f32,
                              kind="ExternalInput")
    # out / rs / partial tensors use a partition-major layout — column block
    # w holds window w's 128 nodes, so per-partition DMA runs are >=512B and
    # dodge the sub-512B half-rate descriptor penalty. Each chunk's partial
    # is [NC*128, |R_k|*128]: the ReduceScatter shards the FLAT input, so the
    # leading 128-row blocks are exactly the per-core sections.
    out_d = nc.dram_tensor("out", [128, cfg.npc], f16, kind="ExternalOutput")
    h_slice_d = nc.dram_tensor("h_slice", [cfg.npc, D], f16)
    pdt = getattr(dt, cfg.partial_dtype)
    chunks = cfg.rs_chunks()
    partial_ds = [
        nc.dram_tensor(f"partial{k}", [NC * 128, (r1 - r0) * 128], pdt)
        for k, (r0, r1) in enumerate(chunks)
    ]
    rs_ds = [
        nc.dram_tensor(f"rs{k}", [128, (r1 - r0) * 128], pdt)
        for k, (r0, r1) in enumerate(chunks)
    ]

    Copy = mybir.ActivationFunctionType.Copy
    is_eq, mult = mybir.AluOpType.is_equal, mybir.AluOpType.mult

    octs, oct_meta = cfg.oct_groups()

    grp_pieces = [[] for _ in plan.octs]
    for pi, (wpos, t, oi) in enumerate(plan.pieces):
        grp_pieces[oi].append((pi, wpos, t))
    grp_ginstrs = [[] for _ in plan.octs]
    for inst in plan.ginstrs:
        grp_ginstrs[inst[3]].append(inst)

    max_oct_tiles = int(plan.T2.max())
    max_oct_pieces = max(len(g) for g in grp_pieces)
    groups1 = [list(range(q, min(q + cfg.group, RPC)))
               for q in range(0, RPC, cfg.group)]
    max_grp1_tiles = max(int(sum(plan.T1[w] for w in g)) for g in groups1)

    with tile.TileContext(nc) as tc, ExitStack() as ctx:
        const = ctx.enter_context(tc.tile_pool(name="const", bufs=1))
        g1pool = ctx.enter_context(tc.tile_pool(name="g1", bufs=cfg.gbufs))
        s1pool = ctx.enter_context(tc.tile_pool(name="s1", bufs=cfg.sbufs))
        g2pool = ctx.enter_context(tc.tile_pool(name="g2", bufs=cfg.gbufs))
        s2pool = ctx.enter_context(tc.tile_pool(name="s2", bufs=cfg.sbufs))
        hpool = ctx.enter_context(tc.tile_pool(name="hw", bufs=4))
        rspool = ctx.enter_context(tc.tile_pool(name="rsl", bufs=1))
        psum_w = ctx.enter_context(tc.tile_pool(name="pw", bufs=6, space="PSUM"))
        psum_r = ctx.enter_context(tc.tile_pool(name="pr", bufs=2, space="PSUM"))

        xT_s = const.tile([D, cfg.npc], f16)
        nc.sync.dma_start(xT_s[:], xT16_d[:, :])
        w1t_s = const.tile([D, D], f16)
        nc.sync.dma_start(w1t_s[:], w1t_d[:, :])
        w2t_s = const.tile([D, D], f16)
        nc.sync.dma_start(w2t_s[:], w2t_d[:, :])
        iota_s = const.tile([128, 128], f16)
        nc.sync.dma_start(iota_s[:], iota_d[:, :])
        smeta1_s = const.tile([128, 2 * nt1], f32)
        nc.sync.dma_start(smeta1_s[:], smeta1_d[:, :])
        smeta2_s = const.tile([128, 2 * npieces], f32)
        nc.sync.dma_start(smeta2_s[:], smeta2_d[:, :])
        gidx_s = const.tile([128, plan.idx_cols], i16)
        nc.sync.dma_start(gidx_s[:], gidx_d[:, :])
        hT_s = const.tile([D, cfg.npc], f16)
        r2_s = const.tile([128, RPC, D], f16)

        qn = [0]

        for _rep in range(repeat):
            # ================= layer 1 (pull, local dst windows) ==========
            for grp in groups1:
                grp_t0 = int(plan.base1[grp[0]])
                n_gt = int(sum(plan.T1[w] for w in grp))
                gw = len(grp)
                gt = g1pool.tile([128, max_grp1_tiles, D], xdt, tag="g1")
                sg = s1pool.tile([128, max_grp1_tiles, D], f16, tag="s1")
                if n_gt > 0:
                    nc.sync.dma_start(
                        gt[:, 0:n_gt, :],
                        xg1_d[:, grp_t0 * D : (grp_t0 + n_gt) * D],
                    )
                    for o in range(n_gt):
                        t_abs = grp_t0 + o
                        eng = (nc.gpsimd if cfg.pool_build_l1 and
                               t_abs % cfg.pool_build_l1 == 0 else nc.vector)
                        eng.tensor_scalar(
                            sg[:, o, :],
                            iota_s[:],
                            smeta1_s[:, t_abs : t_abs + 1],
                            smeta1_s[:, nt1 + t_abs : nt1 + t_abs + 1],
                            is_eq,
                            mult,
                        )
                stage = hpool.tile([128, cfg.group, D], f16, tag="hstage")
                pw = psum_w.tile([128, cfg.group, D], f32, tag="pw")
                for wi, wl in enumerate(grp):
                    nc.tensor.matmul(
                        pw[:, wi, :],
                        xT_s[:, wl * 128 : (wl + 1) * 128],
                        w1t_s[:],
                        start=True,
                        stop=(plan.T1[wl] == 0),
                    )
                    for j in range(int(plan.T1[wl])):
                        o = int(plan.base1[wl]) - grp_t0 + j
                        nc.tensor.matmul(
                            pw[:, wi, :],
                            sg[:, o, :],
                            gt[:, o, :],
                            start=False,
                            stop=(j == int(plan.T1[wl]) - 1),
                        )
                nc.scalar.activation(stage[:, 0:gw, :], pw[:, 0:gw, :], Copy)
                r0, r1 = grp[0] * 128, (grp[-1] + 1) * 128
                nc.scalar.dma_start(
                    h_slice_d[r0:r1, :].rearrange("(w p) d -> p w d", p=128),
                    stage[:, 0:gw, :],
                )

            # hT rebuild (one DMA transpose, after all h stores)
            nc.scalar.dma_start_transpose(hT_s[:], h_slice_d[:, :])

            # r2 = h @ W2.T per own window (PE from hT; overlaps layer 2)
            for r0b in range(0, RPC, 4):
                nb = min(4, RPC - r0b)
                pr = psum_r.tile([128, 4, D], f32, tag="pr")
                for ri in range(nb):
                    r = r0b + ri
                    nc.tensor.matmul(
                        pr[:, ri, :], hT_s[:, r * 128 : (r + 1) * 128], w2t_s[:],
                        start=True, stop=True,
                    )
                nc.scalar.activation(r2_s[:, r0b : r0b + nb, :], pr[:, 0:nb, :],
                                     Copy)

            # ================= layer 2 (push-local, all 392 windows) ======
            for oi, opos in enumerate(plan.octs):
                n_gt = int(plan.T2[oi])
                t0 = int(plan.base2[oi])
                gt = g2pool.tile([128, max_oct_tiles, D], f16, tag="g2")
                sg = s2pool.tile([128, max_oct_pieces, D], f16, tag="s2")
                if n_gt > 0:
                    for (c0, gt0, n_t, _oi) in grp_ginstrs[oi]:
                        nc.gpsimd.dma_gather(
                            gt[:, gt0 - t0 : gt0 - t0 + n_t, :],
                            h_slice_d[:, :],
                            gidx_s[:, c0 : c0 + n_t * 8],
                            n_t * 128,
                            n_t * 128,
                            D,
                            queue_num=qn[0],
                            single_packet=cfg.single_packet,
                        )
                        qn[0] = (qn[0] + 1) % cfg.n_queues
                for k, (pi, wpos, t) in enumerate(grp_pieces[oi]):
                    eng = (nc.gpsimd if cfg.pool_build_l2 and
                           pi % cfg.pool_build_l2 == 0 else nc.vector)
                    eng.tensor_scalar(
                        sg[:, k, :],
                        iota_s[:],
                        smeta2_s[:, pi : pi + 1],
                        smeta2_s[:, npieces + pi : npieces + pi + 1],
                        is_eq,
                        mult,
                    )
                pstage = hpool.tile([128, cfg.oct, D], pdt, tag="pstage")
                for w0 in range(0, len(opos), 4):
                    sub = opos[w0 : w0 + 4]
                    nb = len(sub)
                    wps = [[k for k, (pi, w, t) in enumerate(grp_pieces[oi])
                            if w == wpos] for wpos in sub]
                    all_full = all(wps)
                    pw = psum_w.tile([128, 4, D], f32, tag="pw")
                    for wi, (wpos, wp) in enumerate(zip(sub, wps)):
                        if not wp:
                            nc.vector.memset(pstage[:, w0 + wi, :], 0.0)
                            continue
                        for j, k in enumerate(wp):
                            _pi, _w, t = grp_pieces[oi][k]
                            nc.tensor.matmul(
                                pw[:, wi, :],
                                sg[:, k, :],
                                gt[:, t - t0, :],
                                start=(j == 0),
                                stop=(j == len(wp) - 1),
                            )
                        if not all_full:
                            nc.scalar.activation(pstage[:, w0 + wi, :],
                                                 pw[:, wi, :], Copy)
                    if all_full:
                        nc.scalar.activation(pstage[:, w0 : w0 + nb, :],
                                             pw[:, 0:nb, :], Copy)
                ck, cw, rst, nwo = oct_meta[oi]
                c0 = (rst - chunks[ck][0]) * 128
                nc.scalar.dma_start(
                    partial_ds[ck][cw * 128 : (cw + 1) * 128,
                                   c0 : c0 + nwo * 128].rearrange(
                        "p (w d) -> p w d", d=D),
                    pstage[:, 0:nwo, :],
                )

            # ============== ReduceScatter chunks + root add ===============
            for k, (r0, r1) in enumerate(chunks):
                nc.gpsimd.collective_compute(
                    "ReduceScatter",
                    mybir.AluOpType.add,
                    replica_groups=[list(range(NC))],
                    ins=[partial_ds[k][:, :]],
                    outs=[rs_ds[k][:, :].rearrange("p (w d) -> p w d", d=D)],
                )
            max_nw = max(r1 - r0 for (r0, r1) in chunks)
            for k, (r0, r1) in enumerate(chunks):
                nw_k = r1 - r0
                rsld = rspool.tile([128, max_nw, D], pdt, tag="rsld")
                nc.sync.dma_start(
                    rsld[:, 0:nw_k, :],
                    rs_ds[k][:, :].rearrange("p (w d) -> p w d", d=D),
                )
                ost = rspool.tile([128, max_nw, D], f16, tag="ost")
                nc.vector.tensor_add(ost[:, 0:nw_k, :], rsld[:, 0:nw_k, :],
                                     r2_s[:, r0:r1, :])
                nc.scalar.dma_start(
                    out_d[:, r0 * 128 : r1 * 128].rearrange(
                        "p (w d) -> p w d", d=D),
                    ost[:, 0:nw_k, :],
                )

    nc.compile()
    return nc


_CACHE: dict = {}


def _get_program(plan: Plan):
    key = (
        plan.cfg.n_nodes,
        plan.cfg.n_cores,
        plan.cfg.ranks_per_core,
        tuple(plan.T1.tolist()),
        tuple(plan.T2.tolist()),
        plan.npieces,
    )
    if key not in _CACHE:
        _CACHE[key] = build_program(plan)
    return _CACHE[key]


def kernel(x, edge_index, edge_weight, Wr1, Wr2, cell_len):
    cfg = Cfg()
    assert x.shape == (cfg.n_nodes, D)
    plan, in_maps = preprocess(x, edge_index, edge_weight, Wr1, Wr2, cell_len, cfg)
    nc = _get_program(plan)
    res = run_bass_kernel_spmd(nc, in_maps, list(range(cfg.n_cores)))
    # out is partition-major [128, npc]: node w*128+p lives at [p, w*128:...]
    out = np.concatenate(
        [
            res.results[c]["out"]
            .reshape(128, cfg.ranks_per_core, D)
            .transpose(1, 0, 2)
            .reshape(cfg.npc, D)
            for c in range(cfg.n_cores)
        ],
        axis=0,
    )
    return np.ascontiguousarray(out[: cfg.n_nodes]).astype(np.float32)


# revision 33
# speedup vs baseline: 62.7500x; 4.7562x over previous
"""Trainium2 Bass kernel for a 2-layer GNN message-passing encoder.

Math (per layer):  out = segment_mean(x[src] * w, dst) + x @ Wr.T
with w = typew(src,dst) * edge_weight, run twice (Wr1 then Wr2).

Device strategy (8 NeuronCores, SPMD single program), v3 "push-local L2":

  Layer 1 (pull, dst-partitioned edges): core c owns the contiguous
  6272-node dst range. Per 128-node dst window, the weighted segment-mean
  is a chain of one-hot matmuls accumulated in PSUM:
      S[e, n] = (dst_rel[e] == n) * w'[e],   w' = typew * ew / max(cnt,1)
  where S tiles are built ON DEVICE by one fused DVE tensor_scalar per
  tile from a tiny per-slot metadata stream (rel, w'), and the x[src]
  rows are gathered ON HOST into slot order and streamed as fp8 at byte
  rate (no per-edge descriptors). The root linear x @ W1.T is one more
  matmul into the same PSUM chain (lhsT = resident xT).  h goes to DRAM
  (h_slice, fp16) and hT is rebuilt once with a DMA transpose.

  Layer 2 (push-local, src-partitioned edges): each core computes
  partial aggregates for ALL 392 dst windows from the edges whose SRC it
  owns — h[src] rows are gathered from the core's OWN h_slice with the
  SWDGE dma_gather (local rows < 6272, so int16 indices need no class
  split), messages scatter into per-window PSUM via the same masked
  one-hot matmuls, and partials are stored to DRAM in a chunk-major
  window order.  A chunked ReduceScatter (the only collective) then sums
  partials across cores, delivering each core exactly its own 6272 rows.
  The root linear h @ W2.T is precomputed per own-window into a resident
  r2 buffer (PE, from hT) and added to each ReduceScatter chunk on DVE
  before the final store.  Edge tiles are aligned per OCT (8 windows) so
  per-window padding never hits the gather descriptor stream; windows'
  matmul pieces cover the union of tile spans across cores, with
  per-core masking folded into the per-piece S metadata (zero columns
  contribute nothing).

  Queue discipline: SP carries loads + rs reloads, ACT carries stage
  copies / stores / the hT transpose, Pool carries the SWDGE gathers with
  the ReduceScatters emitted after ALL gathers (so a collective's late
  dependencies never head-of-line block gather issue), and the per-chunk
  root adds sit at the end of the DVE stream behind the S builds — each
  queue's program order matches its readiness order.

  partial/rs/out use a partition-major layout ([128, cols]; column block
  w = window w's nodes) so every DMA touching them moves >=512B runs per
  descriptor, dodging the sub-512B half-rate penalty; each chunk's rs
  output is its own tensor because collective outputs must be contiguous.

Host does index/structure work plus one data-movement precompute (the
layer-1 fp8 gather image); all O(E*D) and O(N*D*D) float math runs on
device.
"""

import sys
from contextlib import ExitStack
from dataclasses import dataclass, field

import numpy as np

sys.path.insert(0, "/opt/trn_rl_repo")

import concourse.bacc as bacc  # noqa: E402
import concourse.mybir as mybir  # noqa: E402
import concourse.tile as tile  # noqa: E402
from concourse.bass_utils import run_bass_kernel_spmd  # noqa: E402

D = 128
SAME_W = 0.3
CROSS_W = 1.0


@dataclass
class Cfg:
    n_nodes: int = 50000
    n_cores: int = 8
    ranks_per_core: int = 49
    group: int = 2            # L1 windows per stream batch
    oct: int = 8              # L2 windows per tile-aligned batch
    # L2 ReduceScatter chunk bounds, exclusive prefix of own-rank index
    rs_bounds: tuple = (16, 32, 44)
    # dtype of the exchanged partial aggregates (fp8 halves store+RS bytes;
    # quantization error averages out across the 8-way reduction)
    partial_dtype: str = "float16"
    # fraction of S one-hot builds issued on GPSIMD (Pool) instead of DVE —
    # Pool is idle in L1 and ~60% idle in L2; its per-op cost is ~2.9x DVE's,
    # so a balanced split shortens the build critical path
    pool_build_l1: int = 4   # 1 of every pool_build_l1 builds goes to Pool
    pool_build_l2: int = 8
    # dtype of the host-gathered layer-1 x[src] image
    xg_dtype: str = "float8e4"
    dma_scratch: int = 32768
    gather_tiles_max: int = 32
    n_queues: int = 4
    single_packet: bool = False
    gbufs: int = 4            # L1 xg / L2 gather tile pool depth
    sbufs: int = 3            # S-tile pool depth

    @property
    def npc(self) -> int:
        return self.ranks_per_core * 128

    @property
    def npad(self) -> int:
        return self.n_cores * self.npc

    def rs_chunks(self) -> list:
        """[(r0, r1)] own-rank ranges per ReduceScatter chunk."""
        bounds = sorted({min(b, self.ranks_per_core) for b in self.rs_bounds}
                        | {self.ranks_per_core})
        out = []
        prev = 0
        for b in bounds:
            if b > prev:
                out.append((prev, b))
                prev = b
        return out

    def worder(self) -> np.ndarray:
        """worder[global_window] = processing position (chunk-major:
        chunk k, then owner core, then rank)."""
        RPC = self.ranks_per_core
        chunks = self.rs_chunks()
        pos = np.empty(self.n_cores * RPC, np.int64)
        p = 0
        for (r0, r1) in chunks:
            for c in range(self.n_cores):
                for r in range(r0, r1):
                    pos[c * RPC + r] = p
                    p += 1
        return pos

    def oct_groups(self):
        """L2 batches: consecutive worder positions, never straddling a
        (chunk, core) run. Returns (octs, meta) with meta[i] =
        (chunk_idx, owner_core, rank_start, n_windows)."""
        octs, meta = [], []
        p = 0
        for k, (r0, r1) in enumerate(self.rs_chunks()):
            for c in range(self.n_cores):
                r = r0
                while r < r1:
                    n = min(self.oct, r1 - r)
                    octs.append(list(range(p, p + n)))
                    meta.append((k, c, r, n))
                    p += n
                    r += n
        return octs, meta


@dataclass
class Plan:
    cfg: Cfg
    # layer 1
    T1: np.ndarray        # [RPC] tiles per local dst window
    base1: np.ndarray     # [RPC] first tile of window
    nt1: int = 0
    # layer 2
    T2: np.ndarray = None        # [n_octs] tiles per oct
    base2: np.ndarray = None     # [n_octs] first tile of oct
    nt2: int = 0
    octs: list = field(default_factory=list)     # list of [worder positions]
    pieces: list = field(default_factory=list)   # (wpos, tile_abs, oct_idx)
    ginstrs: list = field(default_factory=list)  # (col, tile_abs, n_tiles, oct)
    idx_cols: int = 0
    npieces: int = 0


def _make_plan(cfg: Cfg, cnt1: np.ndarray, cnt2oct: np.ndarray,
               spans: np.ndarray, octs: list) -> Plan:
    """cnt1: [cores, RPC] L1 per-window counts.
    cnt2oct: [cores, n_octs] L2 per-oct counts.
    spans: [n_windows(worder-ordered), 2] union tile span per window
    (absolute tile indices, computed by caller)."""
    T1 = np.ceil(cnt1.max(axis=0) / 128).astype(np.int64)
    base1 = np.concatenate([[0], np.cumsum(T1)[:-1]])
    nt1 = int(T1.sum())

    n_octs = cnt2oct.shape[1]
    T2 = np.ceil(cnt2oct.max(axis=0) / 128).astype(np.int64)
    base2 = np.concatenate([[0], np.cumsum(T2)[:-1]])
    nt2 = int(T2.sum())

    pieces = []
    for oi, opos in enumerate(octs):
        for wpos in opos:
            lo, hi = spans[wpos]
            for t in range(lo, hi):
                pieces.append((wpos, int(t), oi))

    ginstrs = []
    col = 0
    for oi in range(n_octs):
        t0, n_run = int(base2[oi]), int(T2[oi])
        done = 0
        while done < n_run:
            n = min(cfg.gather_tiles_max, n_run - done)
            ginstrs.append((col, t0 + done, n, oi))
            col += ((n * 8 + 63) // 64) * 64
            done += n
    return Plan(cfg=cfg, T1=T1, base1=base1, nt1=nt1, T2=T2, base2=base2,
                nt2=nt2, octs=octs, pieces=pieces, ginstrs=ginstrs,
                idx_cols=max(col, 64), npieces=len(pieces))


def preprocess(x, edge_index, edge_weight, Wr1, Wr2, cell_len, cfg: Cfg):
    """Host-side index/structure prep. Returns (plan, in_maps)."""
    RPC = cfg.ranks_per_core
    NC = cfg.n_cores
    src = np.asarray(edge_index[0], dtype=np.int64)
    dst = np.asarray(edge_index[1], dtype=np.int64)
    ew = np.asarray(edge_weight, dtype=np.float32)
    cl = int(np.asarray(cell_len))
    x = np.asarray(x, dtype=np.float32)

    tw = np.where((src > cl) == (dst > cl), SAME_W, CROSS_W).astype(np.float32)
    cnt = np.bincount(dst, minlength=cfg.n_nodes).astype(np.float32)
    inv = (1.0 / np.maximum(cnt, 1.0)).astype(np.float32)
    wfin = tw * ew * inv[dst]

    g_dst = dst >> 7                    # global dst window
    core1 = g_dst // RPC                # L1 owner (by dst)
    wl1 = g_dst - core1 * RPC
    cnt1 = np.bincount(core1 * RPC + wl1, minlength=NC * RPC).reshape(NC, RPC)

    # ---- layer 2 structure (by src core, chunk-major window order) ----
    worder = cfg.worder()               # global window -> position
    wpos_e = worder[g_dst]
    core2 = src // cfg.npc              # L2 owner (by src)
    src_local = src - core2 * cfg.npc

    octs, _oct_meta = cfg.oct_groups()
    n_octs = len(octs)
    nw = NC * RPC
    oct_id_of = np.empty(nw, np.int64)       # worder position -> oct index
    oct_first = np.empty(n_octs, np.int64)   # oct -> first worder position
    for oi, opos in enumerate(octs):
        oct_id_of[opos] = oi
        oct_first[oi] = opos[0]
    oct_e = oct_id_of[wpos_e]
    cnt2oct = np.bincount(core2 * n_octs + oct_e,
                          minlength=NC * n_octs).reshape(NC, n_octs)

    # per-core slot position within oct: sort edges by (core2, wpos, src_local)
    order2 = np.lexsort((src_local, wpos_e, core2))
    key2 = (core2 * n_octs + oct_e)[order2]
    starts2 = np.zeros(NC * n_octs + 1, np.int64)
    np.cumsum(np.bincount(key2, minlength=NC * n_octs), out=starts2[1:])
    pos2 = np.arange(len(src)) - starts2[key2]

    # union tile span per window position across cores
    # per (core, wpos): start/end offsets within oct
    wcnt = np.bincount(core2 * nw + wpos_e, minlength=NC * nw).reshape(NC, nw)
    wend = np.cumsum(wcnt, axis=1)      # per core: cumulative end over wpos
    # reset cumsum at oct boundaries: offset within oct
    oct_start_w = oct_first[oct_id_of[np.arange(nw)]]
    base_at_oct = np.where(oct_start_w > 0, wend[:, oct_start_w - 1], 0)
    w_off_end = wend - base_at_oct      # end offset within oct per (core, wpos)
    w_off_start = w_off_end - wcnt

    plan0_T2 = np.ceil(cnt2oct.max(axis=0) / 128).astype(np.int64)
    plan0_base2 = np.concatenate([[0], np.cumsum(plan0_T2)[:-1]])
    has = wcnt > 0
    start_t = np.where(has, w_off_start // 128, np.iinfo(np.int64).max)
    end_t = np.where(has, (w_off_end + 127) // 128, 0)
    lo_w = start_t.min(axis=0)
    hi_w = end_t.max(axis=0)
    spans = np.zeros((nw, 2), np.int64)
    touched = has.any(axis=0)
    oi_w = oct_id_of
    spans[touched, 0] = plan0_base2[oi_w[touched]] + lo_w[touched]
    spans[touched, 1] = plan0_base2[oi_w[touched]] + hi_w[touched]

    plan = _make_plan(cfg, cnt1, cnt2oct, spans, octs)
    nt1, nt2 = plan.nt1, plan.nt2

    # ---- layer 1 slots (per dst core) ----
    order1 = np.lexsort((src, g_dst))
    gid1_s = g_dst[order1]
    starts1 = np.zeros(NC * RPC + 1, np.int64)
    np.cumsum(np.bincount(gid1_s, minlength=NC * RPC), out=starts1[1:])
    pos1 = np.arange(len(src)) - starts1[gid1_s]
    core1_s = core1[order1]
    wl1_s = wl1[order1]
    slot1 = core1_s * (nt1 * 128) + plan.base1[wl1_s] * 128 + pos1

    rel1 = (dst[order1] & 127).astype(np.float32)
    w1v = wfin[order1]
    total1 = NC * nt1 * 128
    rel1_slot = np.full(total1, -1.0, np.float32)
    rel1_slot[slot1] = rel1
    w1_slot = np.zeros(total1, np.float32)
    w1_slot[slot1] = w1v
    src1_slot = np.zeros(total1, np.int64)
    src1_slot[slot1] = src[order1]

    # smeta1: [cores][128, 2*nt1] f32 (col t = rel, col nt1+t = w')
    rel1_pt = rel1_slot.reshape(NC, nt1, 128).transpose(0, 2, 1)
    w1_pt = w1_slot.reshape(NC, nt1, 128).transpose(0, 2, 1)
    smeta1 = np.concatenate([rel1_pt, w1_pt], axis=2)

    # ---- layer 2 slots (per src core) ----
    oct_s = oct_e[order2]
    slot2 = plan.base2[oct_s] * 128 + pos2     # within-core slot
    core2_s = core2[order2]
    total2_core = nt2 * 128
    rel2_slot = np.zeros((NC, total2_core), np.float32)
    w2_slot = np.zeros((NC, total2_core), np.float32)
    win2_slot = np.full((NC, total2_core), -1, np.int64)
    idx2_slot = np.zeros((NC, total2_core), np.int16)
    rel2_slot[core2_s, slot2] = (dst[order2] & 127).astype(np.float32)
    w2_slot[core2_s, slot2] = wfin[order2]
    win2_slot[core2_s, slot2] = wpos_e[order2]
    idx2_slot[core2_s, slot2] = src_local[order2].astype(np.int16)

    # smeta2: per piece p (wpos, t): rel col masked to this window
    piece_w = np.array([p[0] for p in plan.pieces], np.int64)
    piece_t = np.array([p[1] for p in plan.pieces], np.int64)
    npieces = plan.npieces
    rel2_tiles = rel2_slot.reshape(NC, nt2, 128)
    w2_tiles = w2_slot.reshape(NC, nt2, 128)
    win2_tiles = win2_slot.reshape(NC, nt2, 128)
    # [cores, npieces, 128]
    m = win2_tiles[:, piece_t, :] == piece_w[None, :, None]
    rel_p = np.where(m, rel2_tiles[:, piece_t, :], -1.0)
    w_p = np.where(m, w2_tiles[:, piece_t, :], 0.0)
    smeta2 = np.concatenate(
        [rel_p.transpose(0, 2, 1), w_p.transpose(0, 2, 1)], axis=2
    )  # [cores, 128, 2*npieces]

    # device-layout constants
    np_xdt = mybir.dt.np(getattr(mybir.dt, cfg.xg_dtype))
    xnat = np.zeros((cfg.npad, D), np.float32)
    xnat[: cfg.n_nodes] = x
    x8 = xnat.astype(np_xdt)
    iota16 = np.tile(np.arange(128, dtype=np.float16), (128, 1))
    w1t = np.ascontiguousarray(np.asarray(Wr1, np.float16).T)
    w2t = np.ascontiguousarray(np.asarray(Wr2, np.float16).T)

    in_maps = []
    for c in range(NC):
        # layer-1 gathered rows in slot layout [128, nt1*D] (fp8)
        rows = x8[src1_slot[c * nt1 * 128 : (c + 1) * nt1 * 128]]
        xg1 = np.ascontiguousarray(
            rows.reshape(nt1, 128, D).transpose(1, 0, 2).reshape(128, nt1 * D)
        )
        # gather indices for layer 2
        idx_c = idx2_slot[c]
        g16 = np.zeros((16, plan.idx_cols), np.int16)
        for (c0, t0, n_t, _oi) in plan.ginstrs:
            g16[:, c0 : c0 + n_t * 8] = idx_c[t0 * 128 : (t0 + n_t) * 128].reshape(
                -1, 16
            ).T
        gidx = np.ascontiguousarray(np.tile(g16, (8, 1)))
        xT = np.ascontiguousarray(
            xnat[c * cfg.npc : (c + 1) * cfg.npc].astype(np.float16).T
        )
        in_maps.append({
            "xT16": xT,
            "w1t": w1t,
            "w2t": w2t,
            "iota16": iota16,
            "xg1": xg1,
            "gidx": gidx,
            "smeta1": np.ascontiguousarray(smeta1[c]),
            "smeta2": np.ascontiguousarray(smeta2[c]),
        })
    return plan, in_maps


def build_program(plan: Plan, repeat=1):
    cfg = plan.cfg
    RPC = cfg.ranks_per_core
    NC = cfg.n_cores
    dt = mybir.dt
    f32, f16, i16 = dt.float32, dt.float16, dt.int16
    xdt = getattr(dt, cfg.xg_dtype)
    nt1, nt2, npieces = plan.nt1, plan.nt2, plan.npieces

    nc = bacc.Bacc(
        "TRN2",
        target_bir_lowering=False,
        debug=False,
        num_devices=NC,
        dynamic_dma_scratch_size=cfg.dma_scratch,
        num_swdge_queues=cfg.n_queues,
    )
    xT16_d = nc.dram_tensor("xT16", [D, cfg.npc], f16, kind="ExternalInput")
    w1t_d = nc.dram_tensor("w1t", [D, D], f16, kind="ExternalInput")
    w2t_d = nc.dram_tensor("w2t", [D, D], f16, kind="ExternalInput")
    iota_d = nc.dram_tensor("iota16", [128, 128], f16, kind="ExternalInput")
    xg1_d = nc.dram_tensor("xg1", [128, nt1 * D], xdt, kind="ExternalInput")
    gidx_d = nc.dram_tensor("gidx", [128, plan.idx_cols], i16, kind="ExternalInput")
    smeta1_d = nc.dram_tensor("smeta1", [128, 2 * nt1], f32, kind="ExternalInput")
    smeta2_d = nc.dram_tensor("smeta2", [128, 2 * npieces], f32,
                              kind="ExternalInput")
    # out / rs / partial tensors use a partition-major layout — column block
    # w holds window w's 128 nodes, so per-partition DMA runs are >=512B and
    # dodge the sub-512B half-rate descriptor penalty. Each chunk's partial
    # is [NC*128, |R_k|*128]: the ReduceScatter shards the FLAT input, so the
    # leading 128-row blocks are exactly the per-core sections.
    out_d = nc.dram_tensor("out", [128, cfg.npc], f16, kind="ExternalOutput")
    h_slice_d = nc.dram_tensor("h_slice", [cfg.npc, D], f16)
    pdt = getattr(dt, cfg.partial_dtype)
    chunks = cfg.rs_chunks()
    partial_ds = [
        nc.dram_tensor(f"partial{k}", [NC * 128, (r1 - r0) * 128], pdt)
        for k, (r0, r1) in enumerate(chunks)
    ]
    rs_ds = [
        nc.dram_tensor(f"rs{k}", [128, (r1 - r0) * 128], pdt)
        for k, (r0, r1) in enumerate(chunks)
    ]

    Copy = mybir.ActivationFunctionType.Copy
    is_eq, mult = mybir.AluOpType.is_equal, mybir.AluOpType.mult

    octs, oct_meta = cfg.oct_groups()

    grp_pieces = [[] for _ in plan.octs]
    for pi, (wpos, t, oi) in enumerate(plan.pieces):
        grp_pieces[oi].append((pi, wpos, t))
    grp_ginstrs = [[] for _ in plan.octs]
    for inst in plan.ginstrs:
        grp_ginstrs[inst[3]].append(inst)

    max_oct_tiles = int(plan.T2.max())
    max_oct_pieces = max(len(g) for g in grp_pieces)
    groups1 = [list(range(q, min(q + cfg.group, RPC)))
               for q in range(0, RPC, cfg.group)]
    max_grp1_tiles = max(int(sum(plan.T1[w] for w in g)) for g in groups1)

    with tile.TileContext(nc) as tc, ExitStack() as ctx:
        const = ctx.enter_context(tc.tile_pool(name="const", bufs=1))
        g1pool = ctx.enter_context(tc.tile_pool(name="g1", bufs=cfg.gbufs))
        s1pool = ctx.enter_context(tc.tile_pool(name="s1", bufs=cfg.sbufs))
        g2pool = ctx.enter_context(tc.tile_pool(name="g2", bufs=cfg.gbufs))
        s2pool = ctx.enter_context(tc.tile_pool(name="s2", bufs=cfg.sbufs))
        hpool = ctx.enter_context(tc.tile_pool(name="hw", bufs=4))
        rspool = ctx.enter_context(tc.tile_pool(name="rsl", bufs=1))
        psum_w = ctx.enter_context(tc.tile_pool(name="pw", bufs=6, space="PSUM"))
        psum_r = ctx.enter_context(tc.tile_pool(name="pr", bufs=2, space="PSUM"))

        xT_s = const.tile([D, cfg.npc], f16)
        nc.sync.dma_start(xT_s[:], xT16_d[:, :])
        w1t_s = const.tile([D, D], f16)
        nc.sync.dma_start(w1t_s[:], w1t_d[:, :])
        w2t_s = const.tile([D, D], f16)
        nc.sync.dma_start(w2t_s[:], w2t_d[:, :])
        iota_s = const.tile([128, 128], f16)
        nc.sync.dma_start(iota_s[:], iota_d[:, :])
        smeta1_s = const.tile([128, 2 * nt1], f32)
        nc.sync.dma_start(smeta1_s[:], smeta1_d[:, :])
        smeta2_s = const.tile([128, 2 * npieces], f32)
        nc.sync.dma_start(smeta2_s[:], smeta2_d[:, :])
        gidx_s = const.tile([128, plan.idx_cols], i16)
        nc.sync.dma_start(gidx_s[:], gidx_d[:, :])
        hT_s = const.tile([D, cfg.npc], f16)
        r2_s = const.tile([128, RPC, D], f16)

        qn = [0]

        for _rep in range(repeat):
            # ================= layer 1 (pull, local dst windows) ==========
            for grp in groups1:
                grp_t0 = int(plan.base1[grp[0]])
                n_gt = int(sum(plan.T1[w] for w in grp))
                gw = len(grp)
                gt = g1pool.tile([128, max_grp1_tiles, D], xdt, tag="g1")
                sg = s1pool.tile([128, max_grp1_tiles, D], f16, tag="s1")
                if n_gt > 0:
                    nc.sync.dma_start(
                        gt[:, 0:n_gt, :],
                        xg1_d[:, grp_t0 * D : (grp_t0 + n_gt) * D],
                    )
                    for o in range(n_gt):
                        t_abs = grp_t0 + o
                        eng = (nc.gpsimd if cfg.pool_build_l1 and
                               t_abs % cfg.pool_build_l1 == 0 else nc.vector)
                        eng.tensor_scalar(
                            sg[:, o, :],
                            iota_s[:],
                            smeta1_s[:, t_abs : t_abs + 1],
                            smeta1_s[:, nt1 + t_abs : nt1 + t_abs + 1],
                            is_eq,
                            mult,
                        )
                stage = hpool.tile([128, cfg.group, D], f16, tag="hstage")
                pw = psum_w.tile([128, cfg.group, D], f32, tag="pw")
                for wi, wl in enumerate(grp):
                    nc.tensor.matmul(
                        pw[:, wi, :],
                        xT_s[:, wl * 128 : (wl + 1) * 128],
                        w1t_s[:],
                        start=True,
                        stop=(plan.T1[wl] == 0),
                    )
                    for j in range(int(plan.T1[wl])):
                        o = int(plan.base1[wl]) - grp_t0 + j
                        nc.tensor.matmul(
                            pw[:, wi, :],
                            sg[:, o, :],
                            gt[:, o, :],
                            start=False,
                            stop=(j == int(plan.T1[wl]) - 1),
                        )
                nc.scalar.activation(stage[:, 0:gw, :], pw[:, 0:gw, :], Copy)
                r0, r1 = grp[0] * 128, (grp[-1] + 1) * 128
                nc.scalar.dma_start(
                    h_slice_d[r0:r1, :].rearrange("(w p) d -> p w d", p=128),
                    stage[:, 0:gw, :],
                )

            # hT rebuild (one DMA transpose, after all h stores)
            nc.scalar.dma_start_transpose(hT_s[:], h_slice_d[:, :])

            # r2 = h @ W2.T per own window (PE from hT; overlaps layer 2)
            for r0b in range(0, RPC, 4):
                nb = min(4, RPC - r0b)
                pr = psum_r.tile([128, 4, D], f32, tag="pr")
                for ri in range(nb):
                    r = r0b + ri
                    nc.tensor.matmul(
                        pr[:, ri, :], hT_s[:, r * 128 : (r + 1) * 128], w2t_s[:],
                        start=True, stop=True,
                    )
                nc.scalar.activation(r2_s[:, r0b : r0b + nb, :], pr[:, 0:nb, :],
                                     Copy)

            # ================= layer 2 (push-local, all 392 windows) ======
            for oi, opos in enumerate(plan.octs):
                n_gt = int(plan.T2[oi])
                t0 = int(plan.base2[oi])
                gt = g2pool.tile([128, max_oct_tiles, D], f16, tag="g2")
                sg = s2pool.tile([128, max_oct_pieces, D], f16, tag="s2")
                if n_gt > 0:
                    for (c0, gt0, n_t, _oi) in grp_ginstrs[oi]:
                        nc.gpsimd.dma_gather(
                            gt[:, gt0 - t0 : gt0 - t0 + n_t, :],
                            h_slice_d[:, :],
                            gidx_s[:, c0 : c0 + n_t * 8],
                            n_t * 128,
                            n_t * 128,
                            D,
                            queue_num=qn[0],
                            single_packet=cfg.single_packet,
                        )
                        qn[0] = (qn[0] + 1) % cfg.n_queues
                for k, (pi, wpos, t) in enumerate(grp_pieces[oi]):
                    eng = (nc.gpsimd if cfg.pool_build_l2 and
                           pi % cfg.pool_build_l2 == 0 else nc.vector)
                    eng.tensor_scalar(
                        sg[:, k, :],
                        iota_s[:],
                        smeta2_s[:, pi : pi + 1],
                        smeta2_s[:, npieces + pi : npieces + pi + 1],
                        is_eq,
                        mult,
                    )
                pstage = hpool.tile([128, cfg.oct, D], pdt, tag="pstage")
                for w0 in range(0, len(opos), 4):
                    sub = opos[w0 : w0 + 4]
                    nb = len(sub)
                    wps = [[k for k, (pi, w, t) in enumerate(grp_pieces[oi])
                            if w == wpos] for wpos in sub]
                    all_full = all(wps)
                    pw = psum_w.tile([128, 4, D], f32, tag="pw")
                    for wi, (wpos, wp) in enumerate(zip(sub, wps)):
                        if not wp:
                            nc.vector.memset(pstage[:, w0 + wi, :], 0.0)
                            continue
                        for j, k in enumerate(wp):
                            _pi, _w, t = grp_pieces[oi][k]
                            nc.tensor.matmul(
                                pw[:, wi, :],
                                sg[:, k, :],
                                gt[:, t - t0, :],
                                start=(j == 0),
                                stop=(j == len(wp) - 1),
                            )
                        if not all_full:
                            nc.scalar.activation(pstage[:, w0 + wi, :],
                                                 pw[:, wi, :], Copy)
                    if all_full:
                        nc.scalar.activation(pstage[:, w0 : w0 + nb, :],
                                             pw[:, 0:nb, :], Copy)
                ck, cw, rst, nwo = oct_meta[oi]
                c0 = (rst - chunks[ck][0]) * 128
                nc.scalar.dma_start(
                    partial_ds[ck][cw * 128 : (cw + 1) * 128,
                                   c0 : c0 + nwo * 128].rearrange(
                        "p (w d) -> p w d", d=D),
                    pstage[:, 0:nwo, :],
                )

            # ============== ReduceScatter chunks + root add ===============
            for k, (r0, r1) in enumerate(chunks):
                nc.gpsimd.collective_compute(
                    "ReduceScatter",
                    mybir.AluOpType.add,
                    replica_groups=[list(range(NC))],
                    ins=[partial_ds[k][:, :]],
                    outs=[rs_ds[k][:, :].rearrange("p (w d) -> p w d", d=D)],
                )
            max_nw = max(r1 - r0 for (r0, r1) in chunks)
            for k, (r0, r1) in enumerate(chunks):
                nw_k = r1 - r0
                rsld = rspool.tile([128, max_nw, D], pdt, tag="rsld")
                nc.sync.dma_start(
                    rsld[:, 0:nw_k, :],
                    rs_ds[k][:, :].rearrange("p (w d) -> p w d", d=D),
                )
                ost = rspool.tile([128, max_nw, D], f16, tag="ost")
                nc.vector.tensor_add(ost[:, 0:nw_k, :], rsld[:, 0:nw_k, :],
                                     r2_s[:, r0:r1, :])
                nc.scalar.dma_start(
                    out_d[:, r0 * 128 : r1 * 128].rearrange(
                        "p (w d) -> p w d", d=D),
                    ost[:, 0:nw_k, :],
                )

    nc.compile()
    return nc


_CACHE: dict = {}


def _get_program(plan: Plan):
    key = (
        plan.cfg.n_nodes,
        plan.cfg.n_cores,
        plan.cfg.ranks_per_core,
        tuple(plan.T1.tolist()),
        tuple(plan.T2.tolist()),
        plan.npieces,
    )
    if key not in _CACHE:
        _CACHE[key] = build_program(plan)
    return _CACHE[key]


def kernel(x, edge_index, edge_weight, Wr1, Wr2, cell_len):
    cfg = Cfg()
    assert x.shape == (cfg.n_nodes, D)
    plan, in_maps = preprocess(x, edge_index, edge_weight, Wr1, Wr2, cell_len, cfg)
    nc = _get_program(plan)
    res = run_bass_kernel_spmd(nc, in_maps, list(range(cfg.n_cores)))
    # out is partition-major [128, npc]: node w*128+p lives at [p, w*128:...]
    out = np.concatenate(
        [
            res.results[c]["out"]
            .reshape(128, cfg.ranks_per_core, D)
            .transpose(1, 0, 2)
            .reshape(cfg.npc, D)
            for c in range(cfg.n_cores)
        ],
        axis=0,
    )
    return np.ascontiguousarray(out[: cfg.n_nodes]).astype(np.float32)
